# revision 37
# baseline (speedup 1.0000x reference)
"""Trainium2 Bass kernel for PVT-style spatial-reduction attention.

Reference computation (per batch):
  x_ds = x[:, ::4, ::4]                                  # nearest downsample 192->48
  q    = q_w @ x_ds + q_b                                # 1x1 conv
  d1   = relu(bn1(dwconv2x2_s2_p1(x_ds)))                # 48 -> 25
  kv1  = bn2(sr2_w @ d1)
  kv2  = dwconv3x3_s1_p1(kv1) + lc_b + kv1
  k,v  = split(kv_w @ kv2 + kv_b)
  out  = softmax(q'k/8) @ v  -> reshape [C,48,48] -> nearest upsample x4

Sharding: 8 cores = 4 batches x 2 head-groups (4 heads / 256 ch each).
Each core runs the identical Bass program on its (batch, head-group) shard
and writes its [256,192,192] slab of the output.
"""

import sys

for _p in ("/root/.axon_site/_ro/trn_rl_repo", "/opt/trn_rl_repo"):
    if _p in sys.path:
        sys.path.remove(_p)
    sys.path.insert(0, _p)

import numpy as np


def _ensure_ntff_hook_module():
    """Provide antenv.axon_hooks (NTFF profile hook registry) if the
    resolved antenv package lacks it — needed for trace=True profiling."""
    try:
        import antenv.axon_hooks  # noqa: F401

        return
    except ImportError:
        pass
    try:
        import types

        import antenv

        mod = types.ModuleType("antenv.axon_hooks")
        mod._HOOK = None

        def set_axon_ntff_profile_hook(hook):
            mod._HOOK = hook

        def get_axon_ntff_profile_hook():
            if mod._HOOK is None:
                try:
                    if "/root/.axon_site" not in sys.path:
                        sys.path.append("/root/.axon_site")
                    from trn_agent_boot.trn_boot import (
                        _ntff_profile_via_ctypes,
                    )

                    mod._HOOK = _ntff_profile_via_ctypes(
                        "/opt/axon/libaxon_pjrt.so"
                    )
                except Exception:
                    mod._HOOK = None
            return mod._HOOK

        mod.set_axon_ntff_profile_hook = set_axon_ntff_profile_hook
        mod.get_axon_ntff_profile_hook = get_axon_ntff_profile_hook
        antenv.axon_hooks = mod
        sys.modules["antenv.axon_hooks"] = mod
    except Exception:
        pass


_ensure_ntff_hook_module()

import concourse.bass as bass
import concourse.tile as tile
from concourse import bacc
from concourse import mybir
from concourse.bass_utils import run_bass_kernel_spmd

F32 = mybir.dt.float32
F32R = mybir.dt.float32r
BF16 = mybir.dt.bfloat16
ALU = mybir.AluOpType
ACTF = mybir.ActivationFunctionType

# Problem constants (hardcoded per contract).
C = 512          # channels
H0 = W0 = 192    # full spatial
HD = WD = 48     # downsampled spatial
N = HD * WD      # 2304 queries
HS = WS = 25     # spatially-reduced size after 2x2/s2/p1 dwconv
M = HS * WS      # 625 keys
HPC = 4          # heads per core
CQ = 256         # q/k/v channels per core
NCORES = 8
BN_EPS = 1e-5
SCALE = 0.125    # hd ** -0.5 = 64 ** -0.5

# n-tiles over the 2304 query positions (psum bank = 512 fp32)
NTS = [(0, 512), (512, 512), (1024, 512), (1536, 512), (2048, 256)]
# m-tiles over the 625 key positions (output-partition tiles)
MTS = [(0, 128), (128, 128), (256, 128), (384, 128), (512, 113)]
# free-dim split of the padded 626 kv free dim (fp32r needs even counts)
MP = 626
MFREE = [(0, 512), (512, 114)]


def _build_nc():
    nc = bacc.Bacc("TRN2", target_bir_lowering=False, debug=False)

    xb = nc.dram_tensor("xb", [C, H0, W0], F32, kind="ExternalInput").ap()
    qwT = nc.dram_tensor("qwT", [128, 4, CQ], F32R, kind="ExternalInput").ap()
    qb = nc.dram_tensor("qb", [128, 2], F32, kind="ExternalInput").ap()
    sr2T = nc.dram_tensor("sr2T", [128, 4, C], F32R, kind="ExternalInput").ap()
    kvkT = nc.dram_tensor("kvkT", [128, 4, CQ], F32R, kind="ExternalInput").ap()
    kvvT = nc.dram_tensor("kvvT", [128, 4, CQ], F32R, kind="ExternalInput").ap()
    kvbk = nc.dram_tensor("kvbk", [128, 2], F32, kind="ExternalInput").ap()
    kvbv = nc.dram_tensor("kvbv", [128, CQ], F32, kind="ExternalInput").ap()
    vecs = nc.dram_tensor("vecs", [128, 4, 18], F32, kind="ExternalInput").ap()
    hsel_d = nc.dram_tensor("hsel", [2, 128], F32R, kind="ExternalInput").ap()
    out_d = nc.dram_tensor("out", [CQ, H0, W0], F32, kind="ExternalOutput").ap()

    with tile.TileContext(nc) as tc:
        with nc.allow_low_precision(
            reason="float32r is fp32-width; matmul accumulation stays fp32"
        ):
            _body(tc, xb, qwT, qb, sr2T, kvkT, kvvT, kvbk, kvbv, vecs,
                  hsel_d, out_d)
    nc.compile()
    return nc


def _body(tc, xb, qwT, qb, sr2T, kvkT, kvvT, kvbk, kvbv, vecs, hsel_d, out_d):
    nc = tc.nc
    from contextlib import ExitStack

    with ExitStack() as ctx:
        consts = ctx.enter_context(tc.tile_pool(name="consts", bufs=1))
        qwT_sb = consts.tile([128, 4, CQ], F32R)
        nc.scalar.dma_start(out=qwT_sb, in_=qwT)
        qb_sb = consts.tile([128, 2], F32)
        nc.scalar.dma_start(out=qb_sb, in_=qb)
        sr2T_sb = consts.tile([128, 4, C], F32R)
        nc.scalar.dma_start(out=sr2T_sb, in_=sr2T)
        kvkT_sb = consts.tile([128, 4, CQ], F32R)
        nc.scalar.dma_start(out=kvkT_sb, in_=kvkT)
        kvvT_sb = consts.tile([128, 4, CQ], F32R)
        nc.scalar.dma_start(out=kvvT_sb, in_=kvvT)
        kvbk_sb = consts.tile([128, 2], F32)
        nc.scalar.dma_start(out=kvbk_sb, in_=kvbk)
        kvbv_sb = consts.tile([128, CQ], F32)
        nc.scalar.dma_start(out=kvbv_sb, in_=kvbv)
        vecs_sb = consts.tile([128, 4, 18], F32)
        nc.scalar.dma_start(out=vecs_sb, in_=vecs)
        zsmall = consts.tile([128, 1], F32)
        nc.vector.memset(zsmall, 0.0)
        osmall = consts.tile([128, 1], F32)
        nc.vector.memset(osmall, 1.0)
        ones1 = consts.tile([1, 64], F32R)
        nc.vector.tensor_copy(
            out=ones1, in_=osmall[0:1, :].to_broadcast([1, 64])
        )
        kvkT_b = consts.tile([128, 4, CQ], BF16)
        nc.vector.tensor_copy(out=kvkT_b, in_=kvkT_sb)
        kvvT_b = consts.tile([128, 4, CQ], BF16)
        nc.vector.tensor_copy(out=kvvT_b, in_=kvvT_sb)

        persist = ctx.enter_context(tc.tile_pool(name="persist", bufs=1))
        x_ds = persist.tile([128, 4, HD, WD], F32R)
        q_sb = persist.tile([128, 2, N], BF16)
        k_loc = persist.tile([128, 2, M], BF16)
        vT_sb = persist.tile([128, 5, HPC, 65], BF16)
        d1 = persist.tile([128, 4, MP], F32R)
        d1s = d1[:, :, 0:M].rearrange("p c (h w) -> p c h w", h=HS)

        # zero d1 up front (no data deps -> runs during first load)
        nc.vector.tensor_copy(out=d1, in_=zsmall.to_broadcast([128, 4, MP]))

        # ---- Phase A: load every 4th row of x in half-chunks, subsample
        # cols on-chip, and run the depthwise 2x2/s2 conv taps per chunk so
        # phase C hides entirely under the input DMA ----
        xb_rows = xb.rearrange("c (h f) w -> c h f w", f=4)
        xv = x_ds.rearrange("p c (h t) (w u) -> p c h t w u", t=2, u=2)
        x_flat = x_ds.rearrange("p c h w -> p c (h w)")
        with ExitStack() as actx:
            rows_p = actx.enter_context(tc.tile_pool(name="rows", bufs=3))
            psQ = actx.enter_context(
                tc.tile_pool(name="psQ", bufs=3, space="PSUM")
            )
            for cc in range(4):
                nsub = 4 if cc == 0 else 2
                for sub in range(nsub):
                    hh24 = 48 // nsub
                    h0 = sub * hh24
                    rows = rows_p.tile([128, hh24, W0], F32, tag="rows")
                    nc.sync.dma_start(
                        out=rows,
                        in_=xb_rows[cc * 128:(cc + 1) * 128,
                                    h0:h0 + hh24, 0, :],
                    )
                    rv = rows.rearrange("p h (w f) -> p h w f", f=4)
                    nc.vector.tensor_copy(
                        out=x_ds[:, cc, h0:h0 + hh24, :], in_=rv[:, :, :, 0]
                    )
                # depthwise 2x2/s2 taps for this chunk (DVE, hidden in load)
                for ki in (0, 1):
                    ro = slice(1, 25) if ki == 0 else slice(0, 24)
                    for kj in (0, 1):
                        co = slice(1, 25) if kj == 0 else slice(0, 24)
                        src = xv[:, cc, :, 1 - ki, :, 1 - kj]
                        dst = d1s[:, cc, ro, co]
                        nc.vector.scalar_tensor_tensor(
                            out=dst,
                            in0=src,
                            scalar=vecs_sb[:, cc, ki * 2 + kj:ki * 2 + kj + 1],
                            in1=dst,
                            op0=ALU.mult,
                            op1=ALU.add,
                        )
                # q-projection partials for this chunk (PE idle during load);
                # first chunk seeds q_sb via ACT (+bias), rest accumulate on
                # DVE. Keeps the post-load serial window free of phase B.
                if cc == 3:
                    continue  # deferred until after phase E (keeps the DVE
                    # queue clear for the 3x3 taps on the critical path)
                for mt in range(2):
                    for (n0, nn) in NTS:
                        ps = psQ.tile([128, 512], F32, tag="psQ")
                        nc.tensor.matmul(
                            ps[:, 0:nn],
                            lhsT=qwT_sb[:, cc, mt * 128:(mt + 1) * 128],
                            rhs=x_flat[:, cc, n0:n0 + nn],
                            start=True,
                            stop=True,
                        )
                        if cc == 0:
                            nc.scalar.activation(
                                out=q_sb[:, mt, n0:n0 + nn],
                                in_=ps[:, 0:nn],
                                func=ACTF.Identity,
                                bias=qb_sb[:, mt:mt + 1],
                                scale=1.0,
                            )
                        else:
                            nc.vector.scalar_tensor_tensor(
                                out=q_sb[:, mt, n0:n0 + nn],
                                in0=ps[:, 0:nn],
                                scalar=osmall[:, 0:1],
                                in1=q_sb[:, mt, n0:n0 + nn],
                                op0=ALU.mult,
                                op1=ALU.add,
                            )

        with ExitStack() as pctx:
            psB = pctx.enter_context(
                tc.tile_pool(name="psB", bufs=2, space="PSUM")
            )
            psV = pctx.enter_context(
                tc.tile_pool(name="psV", bufs=2, space="PSUM")
            )

            # ---- Phase C: BN1 + ReLU (taps already accumulated in phase A) ----
            for cc in range(4):
                nc.scalar.activation(
                    out=d1s[:, cc],
                    in_=d1s[:, cc],
                    func=ACTF.Relu,
                    bias=vecs_sb[:, cc, 5:6],
                    scale=vecs_sb[:, cc, 4:5],
                )

            # ---- Phase D: sr2 1x1 conv + BN2 ----
            d1f = d1
            kv1 = persist.tile([128, 4, HS, WS], BF16)
            kv1f = kv1.rearrange("p c h w -> p c (h w)")
            for mt in range(4):
                ps = psB.tile([128, MP], F32, tag="psB")
                for (f0, ff) in MFREE:
                    for cc in range(4):
                        nc.tensor.matmul(
                            ps[:, f0:f0 + ff],
                            lhsT=sr2T_sb[:, cc, mt * 128:(mt + 1) * 128],
                            rhs=d1f[:, cc, f0:f0 + ff],
                            start=(cc == 0),
                            stop=(cc == 3),
                        )
                nc.scalar.activation(
                    out=kv1f[:, mt],
                    in_=ps[:, 0:M],
                    func=ACTF.Identity,
                    bias=vecs_sb[:, mt, 7:8],
                    scale=vecs_sb[:, mt, 6:7],
                )

            # ---- Phase E: depthwise 3x3 pad-1 conv + lc_b + residual ----
            kv2 = persist.tile([128, 4, MP], BF16)
            nc.vector.tensor_copy(
                out=kv2[:, :, M:MP], in_=zsmall.to_broadcast([128, 4, MP - M])
            )
            kv2s = kv2[:, :, 0:M].rearrange("p c (h w) -> p c h w", h=HS)
            for cc in range(4):
                # center tap: kv2 = (w11 + 1) * kv1 + lc_b  (residual folded)
                nc.scalar.activation(
                    out=kv2s[:, cc],
                    in_=kv1[:, cc],
                    func=ACTF.Identity,
                    bias=vecs_sb[:, cc, 17:18],
                    scale=vecs_sb[:, cc, 12:13],
                )
                for ki in range(3):
                    for kj in range(3):
                        if ki == 1 and kj == 1:
                            continue
                        di, dj = ki - 1, kj - 1
                        a0, a1 = max(0, -di), 25 - max(0, di)
                        b0, b1 = max(0, -dj), 25 - max(0, dj)
                        src = kv1[:, cc, a0 + di:a1 + di, b0 + dj:b1 + dj]
                        dst = kv2s[:, cc, a0:a1, b0:b1]
                        s = 8 + ki * 3 + kj
                        nc.vector.scalar_tensor_tensor(
                            out=dst,
                            in0=src,
                            scalar=vecs_sb[:, cc, s:s + 1],
                            in1=dst,
                            op0=ALU.mult,
                            op1=ALU.add,
                        )

            # deferred chunk-3 q-projection partials: PE fills its idle
            # time here while phase E runs on DVE; the DVE adds overlap
            # phase F's matmuls. (Emitting these during the load would put
            # them ahead of the 3x3 taps in the in-order DVE queue.)
            for mt in range(2):
                for (n0, nn) in NTS:
                    ps = psB.tile([128, MP], F32, tag="psB")
                    nc.tensor.matmul(
                        ps[:, 0:nn],
                        lhsT=qwT_sb[:, 3, mt * 128:(mt + 1) * 128],
                        rhs=x_flat[:, 3, n0:n0 + nn],
                        start=True,
                        stop=True,
                    )
                    nc.vector.scalar_tensor_tensor(
                        out=q_sb[:, mt, n0:n0 + nn],
                        in0=ps[:, 0:nn],
                        scalar=osmall[:, 0:1],
                        in1=q_sb[:, mt, n0:n0 + nn],
                        op0=ALU.mult,
                        op1=ALU.add,
                    )

            # ---- Phase F: k and v projections ----
            kv2f = kv2
            for kt in range(2):
                ps = psB.tile([128, MP], F32, tag="psB")
                for (f0, ff) in MFREE:
                    for cc in range(4):
                        nc.tensor.matmul(
                            ps[:, f0:f0 + ff],
                            lhsT=kvkT_b[:, cc, kt * 128:(kt + 1) * 128],
                            rhs=kv2f[:, cc, f0:f0 + ff],
                            start=(cc == 0),
                            stop=(cc == 3),
                        )
                nc.scalar.activation(
                    out=k_loc[:, kt],
                    in_=ps[:, 0:M],
                    func=ACTF.Identity,
                    bias=kvbk_sb[:, kt:kt + 1],
                    scale=1.0,
                )

            # v, produced directly transposed: vT[m, d] (+ ones column)
            nc.vector.tensor_copy(
                out=vT_sb[:, :, :, 64], in_=osmall.to_broadcast([128, 5, HPC])
            )
            kvbv_h = kvbv_sb.rearrange("p (h d) -> p h d", h=HPC)
            for mi, (m0, msz) in enumerate(MTS):
                ps = psV.tile([128, CQ], F32, tag="psV")
                for cc in range(4):
                    nc.tensor.matmul(
                        ps[:msz],
                        lhsT=kv2f[:, cc, m0:m0 + msz],
                        rhs=kvvT_b[:, cc],
                        start=(cc == 0),
                        stop=(cc == 3),
                    )
                nc.vector.tensor_tensor(
                    out=vT_sb[:msz, mi, :, 0:64],
                    in0=ps[:msz].rearrange("p (h d) -> p h d", h=HPC),
                    in1=kvbv_h[:msz],
                    op=ALU.add,
                )

        # ---- Phase G: attention, normalize, upsample, store ----
        # n processed in sixths of 384: oa tiles fit one psum bank, so four
        # buffers pipeline two units deep and PE never stalls on the
        # epilogue; 384 = 8 whole output rows (no bank/row splits needed).
        NSX = 6
        NSZ = 384
        with ExitStack() as gctx:
            oa_pool = gctx.enter_context(
                tc.tile_pool(name="oa", bufs=4, space="PSUM")
            )
            qk_pool = gctx.enter_context(
                tc.tile_pool(name="qk", bufs=3, space="PSUM")
            )
            bc_pool = gctx.enter_context(
                tc.tile_pool(name="bc", bufs=1, space="PSUM")
            )
            e_pool = gctx.enter_context(tc.tile_pool(name="es", bufs=6))
            uw_pool = gctx.enter_context(tc.tile_pool(name="uw", bufs=3))
            r_pool = gctx.enter_context(tc.tile_pool(name="rp", bufs=4))
            bcs_pool = gctx.enter_context(tc.tile_pool(name="bcs", bufs=4))

            for pr in range(2):
                # head pair (2pr, 2pr+1): k/q partitions 0:64 and 64:128 of
                # group pr; output channels pr*128..pr*128+127. Pairing puts
                # the store source on all 128 partitions — a 64-partition
                # source reads through only half the SBUF AXI ports and caps
                # each SDMA engine at ~14GB/s.
                dst4 = out_d[pr * 128:(pr + 1) * 128].rearrange(
                    "c (t a fp two) w -> c t a fp (two w)", t=NSX, fp=2, two=2
                )
                for t6 in range(NSX):
                    t0 = t6 * NSZ
                    oas = []
                    for hh in range(2):
                        h = 2 * pr + hh
                        hp = hh * 64
                        oa = oa_pool.tile([65, NSZ], F32, tag="oa")
                        for mi, (m0, msz) in enumerate(MTS):
                            ps = qk_pool.tile([128, NSZ], F32, tag="qk")
                            nc.tensor.matmul(
                                ps[:msz],
                                lhsT=k_loc[hp:hp + 64, pr, m0:m0 + msz],
                                rhs=q_sb[hp:hp + 64, pr, t0:t0 + NSZ],
                                start=True,
                                stop=True,
                            )
                            e = e_pool.tile([128, NSZ], BF16, tag="es")
                            nc.scalar.activation(
                                out=e[:msz],
                                in_=ps[:msz],
                                func=ACTF.Exp,
                                scale=SCALE,
                            )
                            nc.tensor.matmul(
                                oa,
                                lhsT=vT_sb[:msz, mi, h, :],
                                rhs=e[:msz],
                                start=(mi == 0),
                                stop=(mi == 4),
                            )
                        oas.append(oa)

                    # epilogue: normalize both heads into one 128-partition
                    # uw tile, replicate the row pair, store via 1536B runs
                    uw = uw_pool.tile([128, 8, 2, W0], F32, tag="uw")
                    uw4 = uw.rearrange(
                        "p a two (w f) -> p a two w f", f=4
                    )
                    rt2 = bcs_pool.tile([1, 2, NSZ], F32R, tag="rt2")
                    for hh in range(2):
                        # custom-DVE ops misbehave at partition base > 0:
                        # run over all 65 partitions (rows 0-63 discarded)
                        rp = r_pool.tile([65, NSZ], F32, tag="rp")
                        nc.vector.reciprocal_approx_fast(out=rp, in_=oas[hh])
                        nc.vector.tensor_copy(
                            out=rt2[0:1, hh], in_=rp[64:65]
                        )
                    bcs = bcs_pool.tile([128, NSZ], F32R, tag="bcs")
                    for hh in range(2):
                        bc = bc_pool.tile([64, NSZ], F32, tag="bc")
                        nc.tensor.matmul(
                            bc,
                            lhsT=ones1,
                            rhs=rt2[0:1, hh],
                            start=True,
                            stop=True,
                        )
                        nc.vector.tensor_copy(
                            out=bcs[hh * 64:(hh + 1) * 64], in_=bc
                        )
                    for hh in range(2):
                        oa = oas[hh]
                        pb = hh * 64
                        src0 = oa[0:64].rearrange(
                            "p (a w) -> p a w", w=48
                        ).unsqueeze(3).broadcast_to([64, 8, 48, 4])
                        src1 = bcs[pb:pb + 64].rearrange(
                            "p (a w) -> p a w", w=48
                        ).unsqueeze(3).broadcast_to([64, 8, 48, 4])
                        nc.vector.tensor_tensor(
                            out=uw4[pb:pb + 64, :, 0, :, :],
                            in0=src0,
                            in1=src1,
                            op=ALU.mult,
                        )
                    nc.scalar.activation(
                        out=uw4[:, 0:6, 1],
                        in_=uw4[:, 0:6, 0],
                        func=ACTF.Identity,
                        scale=1.0,
                    )
                    nc.vector.tensor_copy(
                        out=uw4[:, 6:8, 1], in_=uw4[:, 6:8, 0]
                    )

                    # store this sixth (HWDGE via SP; issuing from ACT
                    # stalls its in-order EXP pipeline)
                    uw_f = uw.rearrange("p a two w -> p a (two w)")
                    for fp in range(2):
                        nc.sync.dma_start(
                            out=dst4[:, t6, :, fp, :], in_=uw_f
                        )


_NC_CACHE = None


def _get_nc():
    global _NC_CACHE
    if _NC_CACHE is None:
        _NC_CACHE = _build_nc()
    return _NC_CACHE


def _prep_in_maps(inputs):
    x = np.ascontiguousarray(np.asarray(inputs["x"], dtype=np.float32))
    q_w = np.asarray(inputs["q_w"], dtype=np.float32)
    q_b = np.asarray(inputs["q_b"], dtype=np.float32)
    kv_w = np.asarray(inputs["kv_w"], dtype=np.float32)
    kv_b = np.asarray(inputs["kv_b"], dtype=np.float32)
    sr1_w = np.asarray(inputs["sr1_w"], dtype=np.float32)
    bn1 = [np.asarray(inputs[f"bn1_{t}"], dtype=np.float32) for t in "gbmv"]
    sr2_w = np.asarray(inputs["sr2_w"], dtype=np.float32)
    bn2 = [np.asarray(inputs[f"bn2_{t}"], dtype=np.float32) for t in "gbmv"]
    lc_w = np.asarray(inputs["lc_w"], dtype=np.float32)
    lc_b = np.asarray(inputs["lc_b"], dtype=np.float32)

    def chan_layout(vec_2d):
        # [C, k] -> [128, 4, k] with channel = cc*128 + p
        k = vec_2d.shape[1]
        return np.ascontiguousarray(
            vec_2d.reshape(4, 128, k).transpose(1, 0, 2)
        )

    s1 = bn1[0] / np.sqrt(bn1[3] + BN_EPS)
    b1 = bn1[1] - bn1[2] * s1
    s2 = bn2[0] / np.sqrt(bn2[3] + BN_EPS)
    b2 = bn2[1] - bn2[2] * s2
    lc = lc_w.reshape(C, 9).copy()
    lc[:, 4] += 1.0  # fold residual into center tap
    vecs = np.zeros((C, 18), np.float32)
    vecs[:, 0:4] = sr1_w.reshape(C, 4)
    vecs[:, 4] = s1
    vecs[:, 5] = b1
    vecs[:, 6] = s2
    vecs[:, 7] = b2
    vecs[:, 8:17] = lc
    vecs[:, 17] = lc_b
    vecs_l = chan_layout(vecs)
    sr2T_l = chan_layout(sr2_w.T.copy())  # [in-ch, out-ch] -> [128,4,512]

    in_maps = []
    for b in range(4):
        for hg in range(2):
            sl = slice(hg * CQ, (hg + 1) * CQ)
            qwT = chan_layout(q_w[sl].T.copy())
            qb_l = np.ascontiguousarray(q_b[sl].reshape(2, 128).T)
            kvkT = chan_layout(kv_w[sl].T.copy())
            kvvT = chan_layout(kv_w[C + hg * CQ:C + (hg + 1) * CQ].T.copy())
            kvbk = np.ascontiguousarray(kv_b[sl].reshape(2, 128).T)
            kvbv = np.ascontiguousarray(
                np.broadcast_to(
                    kv_b[C + hg * CQ:C + (hg + 1) * CQ], (128, CQ)
                ).copy()
            )
            hsel = np.zeros((2, 128), np.float32)
            hsel[0, 0:64] = 1.0
            hsel[1, 64:128] = 1.0
            in_maps.append(
                {
                    "xb": x[b],
                    "hsel": hsel,
                    "qwT": qwT,
                    "qb": qb_l,
                    "sr2T": sr2T_l,
                    "kvkT": kvkT,
                    "kvvT": kvvT,
                    "kvbk": kvbk,
                    "kvbv": kvbv,
                    "vecs": vecs_l,
                }
            )
    return in_maps


def run(inputs, trace=False, **spmd_kwargs):
    """Run the SPMD kernel; returns (output, BassKernelResults)."""
    nc = _get_nc()
    in_maps = _prep_in_maps(inputs)
    res = run_bass_kernel_spmd(
        nc, in_maps, core_ids=list(range(NCORES)), trace=trace, **spmd_kwargs
    )
    out = np.empty((4, C, H0, W0), np.float32)
    i = 0
    for b in range(4):
        for hg in range(2):
            out[b, hg * CQ:(hg + 1) * CQ] = res.results[i]["out"]
            i += 1
    return out, res


def kernel(**inputs):
    out, _ = run(inputs, trace=False)
    return out



# revision 38
# speedup vs baseline: 1.1025x; 1.1025x over previous
"""Trainium2 Bass kernel for PVT-style spatial-reduction attention.

Reference computation (per batch):
  x_ds = x[:, ::4, ::4]                                  # nearest downsample 192->48
  q    = q_w @ x_ds + q_b                                # 1x1 conv
  d1   = relu(bn1(dwconv2x2_s2_p1(x_ds)))                # 48 -> 25
  kv1  = bn2(sr2_w @ d1)
  kv2  = dwconv3x3_s1_p1(kv1) + lc_b + kv1
  k,v  = split(kv_w @ kv2 + kv_b)
  out  = softmax(q'k/8) @ v  -> reshape [C,48,48] -> nearest upsample x4

Sharding: 8 cores = 4 batches x 2 head-groups (4 heads / 256 ch each).
Each core runs the identical Bass program on its (batch, head-group) shard
and writes its [256,192,192] slab of the output.
"""

import sys

for _p in ("/root/.axon_site/_ro/trn_rl_repo", "/opt/trn_rl_repo"):
    if _p in sys.path:
        sys.path.remove(_p)
    sys.path.insert(0, _p)

import numpy as np


def _ensure_ntff_hook_module():
    """Provide antenv.axon_hooks (NTFF profile hook registry) if the
    resolved antenv package lacks it — needed for trace=True profiling."""
    try:
        import antenv.axon_hooks  # noqa: F401

        return
    except ImportError:
        pass
    try:
        import types

        import antenv

        mod = types.ModuleType("antenv.axon_hooks")
        mod._HOOK = None

        def set_axon_ntff_profile_hook(hook):
            mod._HOOK = hook

        def get_axon_ntff_profile_hook():
            if mod._HOOK is None:
                try:
                    if "/root/.axon_site" not in sys.path:
                        sys.path.append("/root/.axon_site")
                    from trn_agent_boot.trn_boot import (
                        _ntff_profile_via_ctypes,
                    )

                    mod._HOOK = _ntff_profile_via_ctypes(
                        "/opt/axon/libaxon_pjrt.so"
                    )
                except Exception:
                    mod._HOOK = None
            return mod._HOOK

        mod.set_axon_ntff_profile_hook = set_axon_ntff_profile_hook
        mod.get_axon_ntff_profile_hook = get_axon_ntff_profile_hook
        antenv.axon_hooks = mod
        sys.modules["antenv.axon_hooks"] = mod
    except Exception:
        pass


_ensure_ntff_hook_module()

import concourse.bass as bass
import concourse.tile as tile
from concourse import bacc
from concourse import mybir
from concourse.bass_utils import run_bass_kernel_spmd

F32 = mybir.dt.float32
F32R = mybir.dt.float32r
BF16 = mybir.dt.bfloat16
ALU = mybir.AluOpType
ACTF = mybir.ActivationFunctionType

# Problem constants (hardcoded per contract).
C = 512          # channels
H0 = W0 = 192    # full spatial
HD = WD = 48     # downsampled spatial
N = HD * WD      # 2304 queries
HS = WS = 25     # spatially-reduced size after 2x2/s2/p1 dwconv
M = HS * WS      # 625 keys
HPC = 4          # heads per core
CQ = 256         # q/k/v channels per core
NCORES = 8
BN_EPS = 1e-5
SCALE = 0.125    # hd ** -0.5 = 64 ** -0.5

# n-tiles over the 2304 query positions (psum bank = 512 fp32)
NTS = [(0, 512), (512, 512), (1024, 512), (1536, 512), (2048, 256)]
# m-tiles over the 625 key positions (output-partition tiles)
MTS = [(0, 128), (128, 128), (256, 128), (384, 128), (512, 113)]
# free-dim split of the padded 626 kv free dim (fp32r needs even counts)
MP = 626
MFREE = [(0, 512), (512, 114)]


def _build_nc():
    nc = bacc.Bacc("TRN2", target_bir_lowering=False, debug=False)

    xb = nc.dram_tensor("xb", [C, H0, W0], F32, kind="ExternalInput").ap()
    qwT = nc.dram_tensor("qwT", [128, 4, CQ], F32R, kind="ExternalInput").ap()
    qb = nc.dram_tensor("qb", [128, 2], F32, kind="ExternalInput").ap()
    sr2T = nc.dram_tensor("sr2T", [128, 4, C], F32R, kind="ExternalInput").ap()
    kvkT = nc.dram_tensor("kvkT", [128, 4, CQ], F32R, kind="ExternalInput").ap()
    kvvT = nc.dram_tensor("kvvT", [128, 4, CQ], F32R, kind="ExternalInput").ap()
    kvbk = nc.dram_tensor("kvbk", [128, 2], F32, kind="ExternalInput").ap()
    kvbv = nc.dram_tensor("kvbv", [128, CQ], F32, kind="ExternalInput").ap()
    vecs = nc.dram_tensor("vecs", [128, 4, 18], F32, kind="ExternalInput").ap()
    hsel_d = nc.dram_tensor("hsel", [2, 128], F32R, kind="ExternalInput").ap()
    out_d = nc.dram_tensor("out", [CQ, H0, W0], F32, kind="ExternalOutput").ap()

    with tile.TileContext(nc) as tc:
        with nc.allow_low_precision(
            reason="float32r is fp32-width; matmul accumulation stays fp32"
        ):
            _body(tc, xb, qwT, qb, sr2T, kvkT, kvvT, kvbk, kvbv, vecs,
                  hsel_d, out_d)
    nc.compile()
    return nc


def _body(tc, xb, qwT, qb, sr2T, kvkT, kvvT, kvbk, kvbv, vecs, hsel_d, out_d):
    nc = tc.nc
    from contextlib import ExitStack

    with ExitStack() as ctx:
        consts = ctx.enter_context(tc.tile_pool(name="consts", bufs=1))
        qwT_sb = consts.tile([128, 4, CQ], F32R)
        nc.sync.dma_start(out=qwT_sb, in_=qwT)
        qb_sb = consts.tile([128, 2], F32)
        nc.sync.dma_start(out=qb_sb, in_=qb)
        sr2T_sb = consts.tile([128, 4, C], F32R)
        nc.sync.dma_start(out=sr2T_sb, in_=sr2T)
        kvkT_sb = consts.tile([128, 4, CQ], F32R)
        nc.sync.dma_start(out=kvkT_sb, in_=kvkT)
        kvvT_sb = consts.tile([128, 4, CQ], F32R)
        nc.sync.dma_start(out=kvvT_sb, in_=kvvT)
        kvbk_sb = consts.tile([128, 2], F32)
        nc.sync.dma_start(out=kvbk_sb, in_=kvbk)
        kvbv_sb = consts.tile([128, CQ], F32)
        nc.sync.dma_start(out=kvbv_sb, in_=kvbv)
        vecs_sb = consts.tile([128, 4, 18], F32)
        nc.sync.dma_start(out=vecs_sb, in_=vecs)
        zsmall = consts.tile([128, 1], F32)
        nc.vector.memset(zsmall, 0.0)
        osmall = consts.tile([128, 1], F32)
        nc.vector.memset(osmall, 1.0)
        ones1 = consts.tile([1, 64], F32R)
        nc.vector.tensor_copy(
            out=ones1, in_=osmall[0:1, :].to_broadcast([1, 64])
        )
        kvkT_b = consts.tile([128, 4, CQ], BF16)
        nc.vector.tensor_copy(out=kvkT_b, in_=kvkT_sb)
        kvvT_b = consts.tile([128, 4, CQ], BF16)
        nc.vector.tensor_copy(out=kvvT_b, in_=kvvT_sb)

        persist = ctx.enter_context(tc.tile_pool(name="persist", bufs=1))
        x_ds = persist.tile([128, 4, HD, WD], F32R)
        q_sb = persist.tile([128, 2, N], BF16)
        k_loc = persist.tile([128, 2, M], BF16)
        vT_sb = persist.tile([128, 5, HPC, 65], BF16)
        d1 = persist.tile([128, 4, MP], F32R)
        d1s = d1[:, :, 0:M].rearrange("p c (h w) -> p c h w", h=HS)

        # zero d1 up front (no data deps -> runs during first load)
        nc.vector.tensor_copy(out=d1, in_=zsmall.to_broadcast([128, 4, MP]))

        # ---- Phase A: load every 4th row of x in half-chunks, subsample
        # cols on-chip, and run the depthwise 2x2/s2 conv taps per chunk so
        # phase C hides entirely under the input DMA ----
        xb_rows = xb.rearrange("c (h f) w -> c h f w", f=4)
        xv = x_ds.rearrange("p c (h t) (w u) -> p c h t w u", t=2, u=2)
        x_flat = x_ds.rearrange("p c h w -> p c (h w)")
        with ExitStack() as actx:
            rows_p = actx.enter_context(tc.tile_pool(name="rows", bufs=3))
            psQ = actx.enter_context(
                tc.tile_pool(name="psQ", bufs=3, space="PSUM")
            )
            for cc in range(4):
                nsub = 4 if cc == 0 else 2
                for sub in range(nsub):
                    hh24 = 48 // nsub
                    h0 = sub * hh24
                    rows = rows_p.tile([128, hh24, W0], F32, tag="rows")
                    nc.sync.dma_start(
                        out=rows,
                        in_=xb_rows[cc * 128:(cc + 1) * 128,
                                    h0:h0 + hh24, 0, :],
                    )
                    rv = rows.rearrange("p h (w f) -> p h w f", f=4)
                    nc.vector.tensor_copy(
                        out=x_ds[:, cc, h0:h0 + hh24, :], in_=rv[:, :, :, 0]
                    )
                # depthwise 2x2/s2 taps for this chunk (DVE, hidden in load)
                for ki in (0, 1):
                    ro = slice(1, 25) if ki == 0 else slice(0, 24)
                    for kj in (0, 1):
                        co = slice(1, 25) if kj == 0 else slice(0, 24)
                        src = xv[:, cc, :, 1 - ki, :, 1 - kj]
                        dst = d1s[:, cc, ro, co]
                        nc.vector.scalar_tensor_tensor(
                            out=dst,
                            in0=src,
                            scalar=vecs_sb[:, cc, ki * 2 + kj:ki * 2 + kj + 1],
                            in1=dst,
                            op0=ALU.mult,
                            op1=ALU.add,
                        )
                # q-projection partials for this chunk (PE idle during load);
                # first chunk seeds q_sb via ACT (+bias), rest accumulate on
                # DVE. Keeps the post-load serial window free of phase B.
                if cc == 3:
                    continue  # deferred until after phase E (keeps the DVE
                    # queue clear for the 3x3 taps on the critical path)
                for mt in range(2):
                    for (n0, nn) in NTS:
                        ps = psQ.tile([128, 512], F32, tag="psQ")
                        nc.tensor.matmul(
                            ps[:, 0:nn],
                            lhsT=qwT_sb[:, cc, mt * 128:(mt + 1) * 128],
                            rhs=x_flat[:, cc, n0:n0 + nn],
                            start=True,
                            stop=True,
                        )
                        if cc == 0:
                            nc.scalar.activation(
                                out=q_sb[:, mt, n0:n0 + nn],
                                in_=ps[:, 0:nn],
                                func=ACTF.Identity,
                                bias=qb_sb[:, mt:mt + 1],
                                scale=1.0,
                            )
                        else:
                            nc.vector.scalar_tensor_tensor(
                                out=q_sb[:, mt, n0:n0 + nn],
                                in0=ps[:, 0:nn],
                                scalar=osmall[:, 0:1],
                                in1=q_sb[:, mt, n0:n0 + nn],
                                op0=ALU.mult,
                                op1=ALU.add,
                            )

        with ExitStack() as pctx:
            psB = pctx.enter_context(
                tc.tile_pool(name="psB", bufs=2, space="PSUM")
            )
            psV = pctx.enter_context(
                tc.tile_pool(name="psV", bufs=2, space="PSUM")
            )

            # ---- Phase C: BN1 + ReLU (taps already accumulated in phase A) ----
            for cc in range(4):
                nc.scalar.activation(
                    out=d1s[:, cc],
                    in_=d1s[:, cc],
                    func=ACTF.Relu,
                    bias=vecs_sb[:, cc, 5:6],
                    scale=vecs_sb[:, cc, 4:5],
                )

            # ---- Phase D: sr2 1x1 conv + BN2 ----
            d1f = d1
            kv1 = persist.tile([128, 4, HS, WS], BF16)
            kv1f = kv1.rearrange("p c h w -> p c (h w)")
            for mt in range(4):
                ps = psB.tile([128, MP], F32, tag="psB")
                for (f0, ff) in MFREE:
                    for cc in range(4):
                        nc.tensor.matmul(
                            ps[:, f0:f0 + ff],
                            lhsT=sr2T_sb[:, cc, mt * 128:(mt + 1) * 128],
                            rhs=d1f[:, cc, f0:f0 + ff],
                            start=(cc == 0),
                            stop=(cc == 3),
                        )
                nc.scalar.activation(
                    out=kv1f[:, mt],
                    in_=ps[:, 0:M],
                    func=ACTF.Identity,
                    bias=vecs_sb[:, mt, 7:8],
                    scale=vecs_sb[:, mt, 6:7],
                )

            # ---- Phase E: depthwise 3x3 pad-1 conv + lc_b + residual ----
            kv2 = persist.tile([128, 4, MP], BF16)
            nc.vector.tensor_copy(
                out=kv2[:, :, M:MP], in_=zsmall.to_broadcast([128, 4, MP - M])
            )
            kv2s = kv2[:, :, 0:M].rearrange("p c (h w) -> p c h w", h=HS)
            for cc in range(4):
                # center tap: kv2 = (w11 + 1) * kv1 + lc_b  (residual folded)
                nc.scalar.activation(
                    out=kv2s[:, cc],
                    in_=kv1[:, cc],
                    func=ACTF.Identity,
                    bias=vecs_sb[:, cc, 17:18],
                    scale=vecs_sb[:, cc, 12:13],
                )
                for ki in range(3):
                    for kj in range(3):
                        if ki == 1 and kj == 1:
                            continue
                        di, dj = ki - 1, kj - 1
                        a0, a1 = max(0, -di), 25 - max(0, di)
                        b0, b1 = max(0, -dj), 25 - max(0, dj)
                        src = kv1[:, cc, a0 + di:a1 + di, b0 + dj:b1 + dj]
                        dst = kv2s[:, cc, a0:a1, b0:b1]
                        s = 8 + ki * 3 + kj
                        nc.vector.scalar_tensor_tensor(
                            out=dst,
                            in0=src,
                            scalar=vecs_sb[:, cc, s:s + 1],
                            in1=dst,
                            op0=ALU.mult,
                            op1=ALU.add,
                        )

            # deferred chunk-3 q-projection partials: PE fills its idle
            # time here while phase E runs on DVE; the DVE adds overlap
            # phase F's matmuls. (Emitting these during the load would put
            # them ahead of the 3x3 taps in the in-order DVE queue.)
            for mt in range(2):
                for (n0, nn) in NTS:
                    ps = psB.tile([128, MP], F32, tag="psB")
                    nc.tensor.matmul(
                        ps[:, 0:nn],
                        lhsT=qwT_sb[:, 3, mt * 128:(mt + 1) * 128],
                        rhs=x_flat[:, 3, n0:n0 + nn],
                        start=True,
                        stop=True,
                    )
                    nc.vector.scalar_tensor_tensor(
                        out=q_sb[:, mt, n0:n0 + nn],
                        in0=ps[:, 0:nn],
                        scalar=osmall[:, 0:1],
                        in1=q_sb[:, mt, n0:n0 + nn],
                        op0=ALU.mult,
                        op1=ALU.add,
                    )

            # ---- Phase F: k and v projections ----
            kv2f = kv2
            for kt in range(2):
                ps = psB.tile([128, MP], F32, tag="psB")
                for (f0, ff) in MFREE:
                    for cc in range(4):
                        nc.tensor.matmul(
                            ps[:, f0:f0 + ff],
                            lhsT=kvkT_b[:, cc, kt * 128:(kt + 1) * 128],
                            rhs=kv2f[:, cc, f0:f0 + ff],
                            start=(cc == 0),
                            stop=(cc == 3),
                        )
                nc.scalar.activation(
                    out=k_loc[:, kt],
                    in_=ps[:, 0:M],
                    func=ACTF.Identity,
                    bias=kvbk_sb[:, kt:kt + 1],
                    scale=1.0,
                )

            # v, produced directly transposed: vT[m, d] (+ ones column)
            nc.vector.tensor_copy(
                out=vT_sb[:, :, :, 64], in_=osmall.to_broadcast([128, 5, HPC])
            )
            kvbv_h = kvbv_sb.rearrange("p (h d) -> p h d", h=HPC)
            for mi, (m0, msz) in enumerate(MTS):
                ps = psV.tile([128, CQ], F32, tag="psV")
                for cc in range(4):
                    nc.tensor.matmul(
                        ps[:msz],
                        lhsT=kv2f[:, cc, m0:m0 + msz],
                        rhs=kvvT_b[:, cc],
                        start=(cc == 0),
                        stop=(cc == 3),
                    )
                nc.vector.tensor_tensor(
                    out=vT_sb[:msz, mi, :, 0:64],
                    in0=ps[:msz].rearrange("p (h d) -> p h d", h=HPC),
                    in1=kvbv_h[:msz],
                    op=ALU.add,
                )

        # ---- Phase G: attention, normalize, upsample, store ----
        # n processed in sixths of 384: oa tiles fit one psum bank, so four
        # buffers pipeline two units deep and PE never stalls on the
        # epilogue; 384 = 8 whole output rows (no bank/row splits needed).
        NSX = 6
        NSZ = 384
        with ExitStack() as gctx:
            oa_pool = gctx.enter_context(
                tc.tile_pool(name="oa", bufs=4, space="PSUM")
            )
            qk_pool = gctx.enter_context(
                tc.tile_pool(name="qk", bufs=3, space="PSUM")
            )
            bc_pool = gctx.enter_context(
                tc.tile_pool(name="bc", bufs=1, space="PSUM")
            )
            e_pool = gctx.enter_context(tc.tile_pool(name="es", bufs=6))
            uw_pool = gctx.enter_context(tc.tile_pool(name="uw", bufs=3))
            r_pool = gctx.enter_context(tc.tile_pool(name="rp", bufs=4))
            bcs_pool = gctx.enter_context(tc.tile_pool(name="bcs", bufs=4))

            for pr in range(2):
                # head pair (2pr, 2pr+1): k/q partitions 0:64 and 64:128 of
                # group pr; output channels pr*128..pr*128+127. Pairing puts
                # the store source on all 128 partitions — a 64-partition
                # source reads through only half the SBUF AXI ports and caps
                # each SDMA engine at ~14GB/s.
                dst4 = out_d[pr * 128:(pr + 1) * 128].rearrange(
                    "c (t a fp two) w -> c t a fp (two w)", t=NSX, fp=2, two=2
                )
                for t6 in range(NSX):
                    t0 = t6 * NSZ
                    oas = []
                    for hh in range(2):
                        h = 2 * pr + hh
                        hp = hh * 64
                        oa = oa_pool.tile([65, NSZ], F32, tag="oa")
                        for mi, (m0, msz) in enumerate(MTS):
                            ps = qk_pool.tile([128, NSZ], F32, tag="qk")
                            nc.tensor.matmul(
                                ps[:msz],
                                lhsT=k_loc[hp:hp + 64, pr, m0:m0 + msz],
                                rhs=q_sb[hp:hp + 64, pr, t0:t0 + NSZ],
                                start=True,
                                stop=True,
                            )
                            e = e_pool.tile([128, NSZ], BF16, tag="es")
                            nc.scalar.activation(
                                out=e[:msz],
                                in_=ps[:msz],
                                func=ACTF.Exp,
                                scale=SCALE,
                            )
                            nc.tensor.matmul(
                                oa,
                                lhsT=vT_sb[:msz, mi, h, :],
                                rhs=e[:msz],
                                start=(mi == 0),
                                stop=(mi == 4),
                            )
                        oas.append(oa)

                    # epilogue: normalize both heads into one 128-partition
                    # uw tile, replicate the row pair, store via 1536B runs
                    uw = uw_pool.tile([128, 8, 2, W0], F32, tag="uw")
                    uw4 = uw.rearrange(
                        "p a two (w f) -> p a two w f", f=4
                    )
                    rt2 = bcs_pool.tile([1, 2, NSZ], F32R, tag="rt2")
                    for hh in range(2):
                        # custom-DVE ops misbehave at partition base > 0:
                        # run over all 65 partitions (rows 0-63 discarded)
                        rp = r_pool.tile([65, NSZ], F32, tag="rp")
                        nc.vector.reciprocal_approx_fast(out=rp, in_=oas[hh])
                        nc.vector.tensor_copy(
                            out=rt2[0:1, hh], in_=rp[64:65]
                        )
                    bcs = bcs_pool.tile([128, NSZ], F32R, tag="bcs")
                    for hh in range(2):
                        bc = bc_pool.tile([64, NSZ], F32, tag="bc")
                        nc.tensor.matmul(
                            bc,
                            lhsT=ones1,
                            rhs=rt2[0:1, hh],
                            start=True,
                            stop=True,
                        )
                        nc.vector.tensor_copy(
                            out=bcs[hh * 64:(hh + 1) * 64], in_=bc
                        )
                    for hh in range(2):
                        oa = oas[hh]
                        pb = hh * 64
                        src0 = oa[0:64].rearrange(
                            "p (a w) -> p a w", w=48
                        ).unsqueeze(3).broadcast_to([64, 8, 48, 4])
                        src1 = bcs[pb:pb + 64].rearrange(
                            "p (a w) -> p a w", w=48
                        ).unsqueeze(3).broadcast_to([64, 8, 48, 4])
                        nc.vector.tensor_tensor(
                            out=uw4[pb:pb + 64, :, 0, :, :],
                            in0=src0,
                            in1=src1,
                            op=ALU.mult,
                        )
                    nc.scalar.activation(
                        out=uw4[:, 0:6, 1],
                        in_=uw4[:, 0:6, 0],
                        func=ACTF.Identity,
                        scale=1.0,
                    )
                    nc.vector.tensor_copy(
                        out=uw4[:, 6:8, 1], in_=uw4[:, 6:8, 0]
                    )

                    # store this sixth (HWDGE via SP; issuing from ACT
                    # stalls its in-order EXP pipeline)
                    uw_f = uw.rearrange("p a two w -> p a (two w)")
                    for fp in range(2):
                        nc.sync.dma_start(
                            out=dst4[:, t6, :, fp, :], in_=uw_f
                        )


_NC_CACHE = None


def _get_nc():
    global _NC_CACHE
    if _NC_CACHE is None:
        _NC_CACHE = _build_nc()
    return _NC_CACHE


def _prep_in_maps(inputs):
    x = np.ascontiguousarray(np.asarray(inputs["x"], dtype=np.float32))
    q_w = np.asarray(inputs["q_w"], dtype=np.float32)
    q_b = np.asarray(inputs["q_b"], dtype=np.float32)
    kv_w = np.asarray(inputs["kv_w"], dtype=np.float32)
    kv_b = np.asarray(inputs["kv_b"], dtype=np.float32)
    sr1_w = np.asarray(inputs["sr1_w"], dtype=np.float32)
    bn1 = [np.asarray(inputs[f"bn1_{t}"], dtype=np.float32) for t in "gbmv"]
    sr2_w = np.asarray(inputs["sr2_w"], dtype=np.float32)
    bn2 = [np.asarray(inputs[f"bn2_{t}"], dtype=np.float32) for t in "gbmv"]
    lc_w = np.asarray(inputs["lc_w"], dtype=np.float32)
    lc_b = np.asarray(inputs["lc_b"], dtype=np.float32)

    def chan_layout(vec_2d):
        # [C, k] -> [128, 4, k] with channel = cc*128 + p
        k = vec_2d.shape[1]
        return np.ascontiguousarray(
            vec_2d.reshape(4, 128, k).transpose(1, 0, 2)
        )

    s1 = bn1[0] / np.sqrt(bn1[3] + BN_EPS)
    b1 = bn1[1] - bn1[2] * s1
    s2 = bn2[0] / np.sqrt(bn2[3] + BN_EPS)
    b2 = bn2[1] - bn2[2] * s2
    lc = lc_w.reshape(C, 9).copy()
    lc[:, 4] += 1.0  # fold residual into center tap
    vecs = np.zeros((C, 18), np.float32)
    vecs[:, 0:4] = sr1_w.reshape(C, 4)
    vecs[:, 4] = s1
    vecs[:, 5] = b1
    vecs[:, 6] = s2
    vecs[:, 7] = b2
    vecs[:, 8:17] = lc
    vecs[:, 17] = lc_b
    vecs_l = chan_layout(vecs)
    sr2T_l = chan_layout(sr2_w.T.copy())  # [in-ch, out-ch] -> [128,4,512]

    in_maps = []
    for b in range(4):
        for hg in range(2):
            sl = slice(hg * CQ, (hg + 1) * CQ)
            qwT = chan_layout(q_w[sl].T.copy())
            qb_l = np.ascontiguousarray(q_b[sl].reshape(2, 128).T)
            kvkT = chan_layout(kv_w[sl].T.copy())
            kvvT = chan_layout(kv_w[C + hg * CQ:C + (hg + 1) * CQ].T.copy())
            kvbk = np.ascontiguousarray(kv_b[sl].reshape(2, 128).T)
            kvbv = np.ascontiguousarray(
                np.broadcast_to(
                    kv_b[C + hg * CQ:C + (hg + 1) * CQ], (128, CQ)
                ).copy()
            )
            hsel = np.zeros((2, 128), np.float32)
            hsel[0, 0:64] = 1.0
            hsel[1, 64:128] = 1.0
            in_maps.append(
                {
                    "xb": x[b],
                    "hsel": hsel,
                    "qwT": qwT,
                    "qb": qb_l,
                    "sr2T": sr2T_l,
                    "kvkT": kvkT,
                    "kvvT": kvvT,
                    "kvbk": kvbk,
                    "kvbv": kvbv,
                    "vecs": vecs_l,
                }
            )
    return in_maps


def run(inputs, trace=False, **spmd_kwargs):
    """Run the SPMD kernel; returns (output, BassKernelResults)."""
    nc = _get_nc()
    in_maps = _prep_in_maps(inputs)
    res = run_bass_kernel_spmd(
        nc, in_maps, core_ids=list(range(NCORES)), trace=trace, **spmd_kwargs
    )
    out = np.empty((4, C, H0, W0), np.float32)
    i = 0
    for b in range(4):
        for hg in range(2):
            out[b, hg * CQ:(hg + 1) * CQ] = res.results[i]["out"]
            i += 1
    return out, res


def kernel(**inputs):
    out, _ = run(inputs, trace=False)
    return out

